# revision 1
# baseline (speedup 1.0000x reference)
# CATS-SwiGLU decode kernel for TRN2 (8 NeuronCores, SPMD tensor-parallel).
#
# Reference computation (decode path, B=S=1):
#   x1    = silu(x @ Wgatet)                  [1,1,dff]
#   flags = |x1| > threshold
#   z     = where(flags, (x @ Wup.T) * x1, 0) [1,1,dff]
#   out   = z @ Wdownt                        [1,1,d]
#
# Sharding: d_ff (11008) split across 8 cores (1376 rows each). Each core
# computes its z slice and a full-width partial down-projection; the host
# sums the 8 partials (the all-reduce of the TP hint, done on host).
#
# The gate/up GEMVs and most of the down GEMV run on the Vector engine as
# fused multiply+reduce (affine_mul_reduce) over weight tiles laid out
# rows-on-partitions (host-pretransposed where needed); DVE streams weights
# at ~444 GB/s, around per-core HBM rate, so the kernel is memory-bound at
# full fp32 precision.  z is replicated across partitions on the otherwise
# idle TensorEngine (transpose-matmul to a PSUM row, copy to SBUF,
# broadcast-matmul into PSUM).  The down-projection tail is split: d-chunks
# 0..19 reduce on DVE against the PSUM z_rep; d columns [2560, 4096) are
# computed on the TensorEngine (zm columns as stationary, natural-layout
# Wdownt as moving) so both engines drain the tail concurrently.
import sys

for _p in ("/opt/trn_rl_repo",):
    if _p not in sys.path:
        sys.path.insert(0, _p)

import numpy as np

import concourse.bass as bass
import concourse.tile as tile
from concourse import bacc, mybir
from concourse.bass_utils import run_bass_kernel_spmd
from concourse.masks import make_identity

D = 4096
FF = 11008
NCORES = 8
FSH = FF // NCORES          # 1376 rows of d_ff per core
NCH = (FSH + 127) // 128    # 11 chunks of <=128 rows
LAST = FSH - 128 * (NCH - 1)  # 96 rows in the last chunk
NDG = 16                    # down-proj groups: 2 d-chunks (256 d) each
DPE0 = 2 * NDG * 128        # 2560: first d column of the PE share
DPE = D - DPE0              # 1536 PE-share columns (= 3 x 512)
CSPLIT = 9                  # z batch 1 = chunks [0, 9); batch 2 stays tiny
F32 = mybir.dt.float32

_CACHE = {}


def _bcast(ap, parts):
    """Replicate a 1-D AP across `parts` partitions (0-stride partition dim)."""
    return bass.AP(tensor=ap.tensor, offset=ap.offset, ap=[[0, parts]] + list(ap.ap))


def _build_nc():
    nc = bacc.Bacc("TRN2", target_bir_lowering=False, debug=False)

    x_d = nc.dram_tensor("x", [D], F32, kind="ExternalInput")
    wg_d = nc.dram_tensor("wg", [FSH, D], F32, kind="ExternalInput")
    wu_d = nc.dram_tensor("wu", [FSH, D], F32, kind="ExternalInput")
    wd_d = nc.dram_tensor("wd", [NDG, 128, 2 * FSH], F32, kind="ExternalInput")
    thr_d = nc.dram_tensor("thr", [1], F32, kind="ExternalInput")
    out_d = nc.dram_tensor("out", [128, 2 * NDG], F32, kind="ExternalOutput")

    with tile.TileContext(nc) as tc:
        with (
            tc.tile_pool(name="const", bufs=1) as const_pool,
            tc.tile_pool(name="wpool", bufs=4) as wpool,
            tc.tile_pool(name="apool", bufs=4) as apool,
            tc.tile_pool(name="acts", bufs=1) as acts,
            tc.tile_pool(name="psum", bufs=1, space="PSUM") as psum,
        ):
            # constants on the scalar (qAct) ring so the weight stream on
            # the sync (qSP) ring starts at t=0
            x_rep = const_pool.tile([128, D], F32)
            nc.scalar.dma_start(out=x_rep[:], in_=_bcast(x_d.ap(), 128))
            thr_sb = const_pool.tile([128, 1], F32)
            nc.scalar.dma_start(out=thr_sb[:], in_=_bcast(thr_d.ap(), 128))

            x1 = acts.tile([128, NCH], F32)  # gate pre-activation
            u = acts.tile([128, NCH], F32)   # up projection
            zm = acts.tile([128, NCH], F32)  # masked z
            nc.vector.memset(x1[:], 0.0)
            nc.vector.memset(u[:], 0.0)

            # warm the sigmoid/abs ACT tables while the DMA stream runs
            warm = acts.tile([128, 1], F32)
            nc.scalar.activation(
                warm[:], thr_sb[:], mybir.ActivationFunctionType.Sigmoid
            )
            nc.scalar.activation(
                warm[:], thr_sb[:], mybir.ActivationFunctionType.Abs
            )

            # z replication machinery (TensorEngine)
            ident = const_pool.tile([128, 128], F32)
            make_identity(nc, ident[:])
            ones_row = const_pool.tile([1, 128], F32)
            nc.vector.memset(ones_row[:], 1.0)
            z_row_ps = psum.tile([1, NCH * 128], F32)
            z_row_sb = const_pool.tile([1, NCH * 128], F32)
            z_rep = psum.tile([128, NCH * 128], F32)
            batches = ((0, CSPLIT), (CSPLIT, NCH))

            def z_batch_compute(bi):
                c0, c1 = batches[bi]
                cs = slice(c0, c1)
                sg = acts.tile([128, NCH], F32, tag="sg", name="sg")
                nc.scalar.activation(
                    sg[:, cs], x1[:, cs], mybir.ActivationFunctionType.Sigmoid
                )
                x1s = acts.tile([128, NCH], F32, tag="x1s", name="x1s")
                nc.vector.tensor_mul(x1s[:, cs], x1[:, cs], sg[:, cs])
                absx = acts.tile([128, NCH], F32, tag="absx", name="absx")
                nc.scalar.activation(
                    absx[:, cs], x1s[:, cs], mybir.ActivationFunctionType.Abs
                )
                mask = acts.tile([128, NCH], F32, tag="mask", name="mask")
                nc.vector.tensor_scalar(
                    out=mask[:, cs],
                    in0=absx[:, cs],
                    scalar1=thr_sb[:],
                    scalar2=None,
                    op0=mybir.AluOpType.is_gt,
                )
                z = acts.tile([128, NCH], F32, tag="z", name="z")
                nc.vector.tensor_mul(z[:, cs], u[:, cs], x1s[:, cs])
                nc.vector.tensor_mul(zm[:, cs], z[:, cs], mask[:, cs])

            def z_batch_rep(bi):
                c0, c1 = batches[bi]
                for c in range(c0, c1):
                    pc = 128 if c < NCH - 1 else LAST
                    fs = slice(c * 128, c * 128 + pc)
                    nc.tensor.matmul(
                        out=z_row_ps[0:1, fs],
                        lhsT=zm[:pc, c : c + 1],
                        rhs=ident[:pc, :pc],
                        start=True,
                        stop=True,
                    )
                    nc.scalar.copy(z_row_sb[0:1, fs], z_row_ps[0:1, fs])
                    nc.tensor.matmul(
                        out=z_rep[:, fs],
                        lhsT=ones_row[0:1, :],
                        rhs=z_row_sb[0:1, fs],
                        start=True,
                        stop=True,
                    )

            # gate and up GEMVs: acc[p, c] = sum_j W[c*128+p, j] * x[j].
            # All weight DMAs stay back-to-back on the sync ring; batch-1 z
            # compute + PE replication are emitted mid-up-loop so they
            # overlap the stream.
            for wi, (wdram, acc) in enumerate(((wg_d, x1), (wu_d, u))):
                for c in range(NCH):
                    p = 128 if c < NCH - 1 else LAST
                    wt = wpool.tile([128, D], F32, tag="w", name="wt")
                    nc.sync.dma_start(
                        out=wt[:p, :], in_=wdram.ap()[c * 128 : c * 128 + p, :]
                    )
                    nc.vector.affine_mul_reduce(
                        out=wt[:p, :],
                        accum_out=acc[:p, c : c + 1],
                        in0=wt[:p, :],
                        in1=x_rep[:p, :],
                        scale=1.0,
                        bias=0.0,
                    )
                    if wi == 1 and c == CSPLIT - 1:
                        z_batch_compute(0)
                        z_batch_rep(0)
            z_batch_compute(1)
            z_batch_rep(1)

            # down projection: osb[p, c] = sum_f WdT[c*128+p, f] * z[f]
            osb = acts.tile([128, 2 * NDG], F32)
            for g in range(NDG):
                dt_ = apool.tile([128, 2 * FSH], F32, tag="wd", name="dt_")
                nc.scalar.dma_start(out=dt_[:], in_=wd_d.ap()[g])
                for h in range(2):
                    sl = slice(h * FSH, (h + 1) * FSH)
                    nc.vector.affine_mul_reduce(
                        out=dt_[:, sl],
                        accum_out=osb[:, 2 * g + h : 2 * g + h + 1],
                        in0=dt_[:, sl],
                        in1=z_rep[:, 0:FSH],
                        scale=1.0,
                        bias=0.0,
                    )

            nc.sync.dma_start(out=out_d.ap(), in_=osb[:])

    nc.compile()
    return nc


def _get_nc():
    if "nc" not in _CACHE:
        _CACHE["nc"] = _build_nc()
    return _CACHE["nc"]


def make_in_maps(x, Wup, Wgatet, Wdownt, threshold):
    """Shard full inputs into the 8 per-core input maps."""
    x_flat = np.ascontiguousarray(np.asarray(x, dtype=np.float32).reshape(D))
    thr = np.asarray(threshold, dtype=np.float32).reshape(1)
    Wup = np.asarray(Wup, dtype=np.float32)
    Wgatet = np.asarray(Wgatet, dtype=np.float32)
    Wdownt = np.asarray(Wdownt, dtype=np.float32)
    in_maps = []
    for i in range(NCORES):
        sl = slice(i * FSH, (i + 1) * FSH)
        wg = np.ascontiguousarray(Wgatet[:, sl].T)          # [FSH, D]
        wu = np.ascontiguousarray(Wup[sl, :])               # [FSH, D]
        wdt = np.ascontiguousarray(Wdownt[sl, :].T)         # [D, FSH]
        a = wdt.reshape(2 * NDG, 128, FSH)
        wd = np.ascontiguousarray(
            np.concatenate([a[0::2], a[1::2]], axis=2)
        )                                                   # [NDG, 128, 2*FSH]
        in_maps.append({"x": x_flat, "wg": wg, "wu": wu, "wd": wd, "thr": thr})
    return in_maps


def run_sharded(x, Wup, Wgatet, Wdownt, threshold, trace=False, tmpdir=None):
    """Run on the 8 NeuronCores; returns (full_output, BassKernelResults)."""
    nc = _get_nc()
    in_maps = make_in_maps(x, Wup, Wgatet, Wdownt, threshold)
    res = run_bass_kernel_spmd(
        nc, in_maps, list(range(NCORES)), trace=trace, tmpdir=tmpdir
    )
    # un-shard: osb[p, c] holds partial_out[c*128 + p]; sum partials over cores
    acc = np.zeros(D, dtype=np.float64)
    for r in res.results:
        acc += r["out"].T.reshape(D).astype(np.float64)
    out = acc.astype(np.float32).reshape(1, 1, D)
    return out, res


def kernel(x, Wup, Wgatet, Wdownt, threshold):
    out, _ = run_sharded(x, Wup, Wgatet, Wdownt, threshold)
    return out



# revision 10
# speedup vs baseline: 1.6701x; 1.6701x over previous
# CATS-SwiGLU decode kernel for TRN2 (8 NeuronCores, SPMD tensor-parallel).
#
# Reference computation (decode path, B=S=1):
#   x1    = silu(x @ Wgatet)                  [1,1,dff]
#   flags = |x1| > threshold
#   z     = where(flags, (x @ Wup.T) * x1, 0) [1,1,dff]
#   out   = z @ Wdownt                        [1,1,d]
#
# Sharding: d_ff (11008) split across 8 cores (1376 rows each). Each core
# computes its z slice and a full-width partial down-projection; the host
# sums the 8 partials (the all-reduce of the TP hint, done on host).
#
# All three weight matrices are cast to fp16 on the host (rel err ~2.4e-3,
# tolerance 2e-2), halving HBM traffic vs fp32 — the kernel is DMA-bound, so
# this halves runtime.  Engine split so every engine has >=2x slack vs the
# ~98us weight stream:
#   - gate GEMV on DVE: affine_mul_reduce over [128,4096] fp16 chunks
#     (f-rows on partitions) against an on-chip-replicated x; x1 lands
#     directly in [128,11] chunk-column layout.
#   - up GEMV on PE: Wup host-transposed to [4096,1376] (k on partitions),
#     x k-chunks as stationary, fp16 1 cycle/row; accumulates in PSUM.
#   - z-chain ([128,11] elementwise) on DVE/Act; u transposed to the
#     [128,11] layout with 11 tiny ones-matmuls.
#   - down-projection on PE: Wdownt natural [1376,4096] rows-on-partitions
#     chunks as moving tensor, zm fp16 columns as stationary, accumulating
#     [8,512] d-tiles in PSUM.
import sys

for _p in ("/opt/trn_rl_repo",):
    if _p not in sys.path:
        sys.path.insert(0, _p)

import numpy as np

import concourse.bass as bass
import concourse.tile as tile
from concourse import bacc, mybir
from concourse.bass_utils import run_bass_kernel_spmd

D = 4096
FF = 11008
NCORES = 8
FSH = FF // NCORES          # 1376 rows of d_ff per core
FC = (FSH + 127) // 128     # 11 f-chunks of <=128 rows
FLAST = FSH - 128 * (FC - 1)  # 96 rows in the last f-chunk
KC = D // 128               # 32 k-chunks for the up GEMV
NTS = ((0, 512), (512, 1024), (1024, FSH))  # f n-tiles for up-GEMV PSUM rows
NDT = 8                     # down-proj d-tiles of 512
F32 = mybir.dt.float32
F16 = mybir.dt.float16

_CACHE = {}


def _bcast(ap, parts):
    """Replicate a 1-D AP across `parts` partitions (0-stride partition dim)."""
    return bass.AP(tensor=ap.tensor, offset=ap.offset, ap=[[0, parts]] + list(ap.ap))


def _build_nc():
    nc = bacc.Bacc("TRN2", target_bir_lowering=False, debug=False)

    xf_d = nc.dram_tensor("xf", [D], F16, kind="ExternalInput")
    xc_d = nc.dram_tensor("xc", [128, KC], F16, kind="ExternalInput")
    wg_d = nc.dram_tensor("wg", [FSH, D], F16, kind="ExternalInput")
    wu_d = nc.dram_tensor("wu", [D, FSH], F16, kind="ExternalInput")
    wd_d = nc.dram_tensor("wd", [FSH, D], F16, kind="ExternalInput")
    thr_d = nc.dram_tensor("thr", [1], F32, kind="ExternalInput")
    out_d = nc.dram_tensor("out", [1, D], F32, kind="ExternalOutput")

    with tile.TileContext(nc) as tc:
        with (
            tc.tile_pool(name="const", bufs=1) as const_pool,
            tc.tile_pool(name="gpool", bufs=6) as gpool,
            tc.tile_pool(name="upool", bufs=6) as upool,
            tc.tile_pool(name="dpool", bufs=4) as dpool,
            tc.tile_pool(name="acts", bufs=1) as acts,
            tc.tile_pool(name="psum", bufs=1, space="PSUM") as psum,
        ):
            # constants on the scalar (qAct) ring so the weight stream on
            # the sync (qSP) ring starts at t=0
            xf_sb = const_pool.tile([1, D], F16)
            nc.scalar.dma_start(out=xf_sb[:], in_=_bcast(xf_d.ap(), 1))
            xc_sb = const_pool.tile([128, KC], F16)
            nc.scalar.dma_start(out=xc_sb[:], in_=xc_d.ap())
            thr_sb = const_pool.tile([128, 1], F32)
            nc.scalar.dma_start(out=thr_sb[:], in_=_bcast(thr_d.ap(), 128))

            ones16 = const_pool.tile([1, 128], F16)
            nc.vector.memset(ones16[:], 1.0)
            x_rep = const_pool.tile([128, D], F32)
            u_sb = const_pool.tile([1, 3 * 512], F16)
            zm16 = const_pool.tile([128, FC], F16)

            x1 = acts.tile([128, FC], F32)   # gate pre-activation accum
            nc.vector.memset(x1[:], 0.0)

            # warm the sigmoid/abs ACT tables while the DMA stream runs
            warm = acts.tile([128, 1], F32)
            nc.scalar.activation(
                warm[:], thr_sb[:], mybir.ActivationFunctionType.Sigmoid
            )
            nc.scalar.activation(
                warm[:], thr_sb[:], mybir.ActivationFunctionType.Abs
            )

            # PSUM tiles.  PE matmul outputs may only start at partition
            # 0/32/64, so the up-GEMV f-tiles live at rows 0/32/64 of one
            # bank and the 8 down-proj d-tiles at rows 0/32/64 of 3 banks.
            u_ps = psum.tile([128, 512], F32)
            nc.vector.memset(u_ps[:], 0.0)
            uT_ps = psum.tile([128, FC], F32)
            nc.vector.memset(uT_ps[:], 0.0)
            o_ps = [
                psum.tile([128, 512], F32, tag=f"o{t}", name=f"o{t}")
                for t in range(3)
            ]

            # replicate x across the 128 partitions on the (otherwise idle)
            # TensorEngine: out[p, j] = sum_{k=1} ones[0, p] * x[0, j]
            for s in range(8):
                xb = psum.tile([128, 512], F32, tag=f"xb{s % 2}", name="xb")
                nc.tensor.matmul(
                    out=xb[:],
                    lhsT=ones16[0:1, :],
                    rhs=xf_sb[0:1, s * 512 : (s + 1) * 512],
                    start=True,
                    stop=True,
                )
                nc.scalar.copy(x_rep[:, s * 512 : (s + 1) * 512], xb[:])

            # gate GEMV on DVE: x1[p, c] = sum_j Wg[c*128+p, j] * x[j]
            for c in range(FC):
                p = 128 if c < FC - 1 else FLAST
                gt = gpool.tile([128, D], F16, tag="g", name="gt")
                nc.sync.dma_start(
                    out=gt[:p, :], in_=wg_d.ap()[c * 128 : c * 128 + p, :]
                )
                nc.vector.affine_mul_reduce(
                    out=gt[:p, :],
                    accum_out=x1[:p, c : c + 1],
                    in0=gt[:p, :],
                    in1=x_rep[:p, :],
                    scale=1.0,
                    bias=0.0,
                )

            # silu + threshold mask, all [128, FC]
            sg = acts.tile([128, FC], F32, tag="sg", name="sg")
            nc.scalar.activation(
                sg[:], x1[:], mybir.ActivationFunctionType.Sigmoid
            )
            x1s = acts.tile([128, FC], F32, tag="x1s", name="x1s")
            nc.vector.tensor_mul(x1s[:], x1[:], sg[:])
            absx = acts.tile([128, FC], F32, tag="absx", name="absx")
            nc.scalar.activation(
                absx[:], x1s[:], mybir.ActivationFunctionType.Abs
            )
            mask = acts.tile([128, FC], F32, tag="mask", name="mask")
            nc.vector.tensor_scalar(
                out=mask[:],
                in0=absx[:],
                scalar1=thr_sb[:],
                scalar2=None,
                op0=mybir.AluOpType.is_gt,
            )
            x1sm = acts.tile([128, FC], F32, tag="x1sm", name="x1sm")
            nc.vector.tensor_mul(x1sm[:], x1s[:], mask[:])

            # up GEMV on PE: u[f] = sum_k x[k] * Wup.T[k, f], f-tiles in PSUM
            for kc in range(KC):
                ut = upool.tile([128, FSH], F16, tag="u", name="ut")
                nc.sync.dma_start(
                    out=ut[:], in_=wu_d.ap()[kc * 128 : (kc + 1) * 128, :]
                )
                for nt, (n0, n1) in enumerate(NTS):
                    nc.tensor.matmul(
                        out=u_ps[32 * nt : 32 * nt + 1, 0 : n1 - n0],
                        lhsT=xc_sb[:, kc : kc + 1],
                        rhs=ut[:, n0:n1],
                        start=(kc == 0),
                        stop=(kc == KC - 1),
                    )

            # u back to [128, FC] chunk-column layout: cast the 3 f-tile rows
            # into one f16 [1, 1376] row, then 11 ones-matmul row->column
            # transposes (f = flat column index)
            for nt, (n0, n1) in enumerate(NTS):
                nc.scalar.copy(
                    u_sb[0:1, n0:n1], u_ps[32 * nt : 32 * nt + 1, 0 : n1 - n0]
                )
            for c in range(FC):
                p = 128 if c < FC - 1 else FLAST
                nc.tensor.matmul(
                    out=uT_ps[0:p, c : c + 1],
                    lhsT=u_sb[0:1, c * 128 : c * 128 + p],
                    rhs=ones16[0:1, 0:1],
                    start=True,
                    stop=True,
                )

            # z = u * silu(x1) * flags, cast to f16 for the PE down-proj
            zm = acts.tile([128, FC], F32, tag="zm", name="zm")
            nc.vector.tensor_mul(zm[:], uT_ps[:], x1sm[:])
            nc.scalar.copy(zm16[:, 0 : FC - 1], zm[:, 0 : FC - 1])
            nc.scalar.copy(zm16[0:FLAST, FC - 1 : FC], zm[0:FLAST, FC - 1 : FC])

            # down projection on PE: o[dt, j] += sum_p zm[c*128+p] * Wd[c*128+p, dt*512+j]
            for c in range(FC):
                p = 128 if c < FC - 1 else FLAST
                dt_ = dpool.tile([128, D], F16, tag="d", name="dt_")
                nc.sync.dma_start(
                    out=dt_[:p, :], in_=wd_d.ap()[c * 128 : c * 128 + p, :]
                )
                for nt in range(NDT):
                    r = 32 * (nt % 3)
                    nc.tensor.matmul(
                        out=o_ps[nt // 3][r : r + 1, :],
                        lhsT=zm16[0:p, c : c + 1],
                        rhs=dt_[0:p, nt * 512 : (nt + 1) * 512],
                        start=(c == 0),
                        stop=(c == FC - 1),
                    )

            osb = const_pool.tile([1, D], F32)
            for nt in range(NDT):
                r = 32 * (nt % 3)
                nc.scalar.copy(
                    osb[0:1, nt * 512 : (nt + 1) * 512],
                    o_ps[nt // 3][r : r + 1, :],
                )
            nc.sync.dma_start(out=out_d.ap(), in_=osb[0:1, :])

    nc.compile()
    return nc


def _get_nc():
    if "nc" not in _CACHE:
        _CACHE["nc"] = _build_nc()
    return _CACHE["nc"]


def make_in_maps(x, Wup, Wgatet, Wdownt, threshold):
    """Shard full inputs into the 8 per-core input maps (fp16 weights)."""
    x_flat = np.asarray(x, dtype=np.float32).reshape(D)
    xf = np.ascontiguousarray(x_flat.astype(np.float16))
    xc = np.ascontiguousarray(xf.reshape(KC, 128).T)        # [128, KC]
    thr = np.asarray(threshold, dtype=np.float32).reshape(1)
    Wup = np.asarray(Wup)
    Wgatet = np.asarray(Wgatet)
    Wdownt = np.asarray(Wdownt)
    in_maps = []
    for i in range(NCORES):
        sl = slice(i * FSH, (i + 1) * FSH)
        wg = np.ascontiguousarray(Wgatet[:, sl].T, dtype=np.float16)  # [FSH, D]
        wu = np.ascontiguousarray(Wup[sl, :].T, dtype=np.float16)     # [D, FSH]
        wd = np.ascontiguousarray(Wdownt[sl, :], dtype=np.float16)    # [FSH, D]
        in_maps.append(
            {"xf": xf, "xc": xc, "wg": wg, "wu": wu, "wd": wd, "thr": thr}
        )
    return in_maps


def run_sharded(x, Wup, Wgatet, Wdownt, threshold, trace=False, tmpdir=None):
    """Run on the 8 NeuronCores; returns (full_output, BassKernelResults)."""
    nc = _get_nc()
    in_maps = make_in_maps(x, Wup, Wgatet, Wdownt, threshold)
    res = run_bass_kernel_spmd(
        nc, in_maps, list(range(NCORES)), trace=trace, tmpdir=tmpdir
    )
    # un-shard: out[nt, j] holds partial_out[nt*512 + j]; sum partials
    acc = np.zeros(D, dtype=np.float64)
    for r in res.results:
        acc += r["out"].reshape(D).astype(np.float64)
    out = acc.astype(np.float32).reshape(1, 1, D)
    return out, res


def kernel(x, Wup, Wgatet, Wdownt, threshold):
    out, _ = run_sharded(x, Wup, Wgatet, Wdownt, threshold)
    return out


# revision 17
# speedup vs baseline: 1.9115x; 1.1446x over previous
# CATS-SwiGLU decode kernel for TRN2 (8 NeuronCores, SPMD tensor-parallel).
#
# Reference computation (decode path, B=S=1):
#   x1    = silu(x @ Wgatet)                  [1,1,dff]
#   flags = |x1| > threshold
#   z     = where(flags, (x @ Wup.T) * x1, 0) [1,1,dff]
#   out   = z @ Wdownt                        [1,1,d]
#
# Sharding: d_ff (11008) split across 8 cores (1376 rows each). Each core
# computes its z slice and a full-width partial down-projection; the host
# sums the 8 partials (the all-reduce of the TP hint, done on host).
#
# All three weight matrices are cast to fp16 on the host (rel err ~2.4e-3,
# tolerance 2e-2), halving HBM traffic vs fp32 — the kernel is DMA-bound, so
# this halves runtime.  Engine split so every engine has >=2x slack vs the
# ~98us weight stream:
#   - gate GEMV on DVE: affine_mul_reduce over [128,4096] fp16 chunks
#     (f-rows on partitions) against an on-chip-replicated x; x1 lands
#     directly in [128,11] chunk-column layout.
#   - up GEMV on PE: Wup host-transposed to [4096,1376] (k on partitions),
#     x k-chunks as stationary, fp16 1 cycle/row; accumulates in PSUM.
#   - z-chain ([128,11] elementwise) on DVE/Act; u transposed to the
#     [128,11] layout with 11 tiny ones-matmuls.
#   - down-projection on PE: Wdownt natural [1376,4096] rows-on-partitions
#     chunks as moving tensor, zm fp16 columns as stationary, accumulating
#     [8,512] d-tiles in PSUM.
import sys

for _p in ("/opt/trn_rl_repo",):
    if _p not in sys.path:
        sys.path.insert(0, _p)

import numpy as np

import concourse.bass as bass
import concourse.tile as tile
from concourse import bacc, mybir
from concourse.bass_utils import run_bass_kernel_spmd

D = 4096
FF = 11008
NCORES = 8
FSH = FF // NCORES          # 1376 rows of d_ff per core
FC = (FSH + 127) // 128     # 11 f-chunks of <=128 rows
FLAST = FSH - 128 * (FC - 1)  # 96 rows in the last f-chunk
KC = D // 128               # 32 k-chunks for the up GEMV
NTS = ((0, 512), (512, 1024), (1024, FSH))  # f n-tiles for up-GEMV PSUM rows
NDT = 8                     # down-proj d-tiles of 512
F32 = mybir.dt.float32
F16 = mybir.dt.float16

_CACHE = {}


def _bcast(ap, parts):
    """Replicate a 1-D AP across `parts` partitions (0-stride partition dim)."""
    return bass.AP(tensor=ap.tensor, offset=ap.offset, ap=[[0, parts]] + list(ap.ap))


def _build_nc():
    nc = bacc.Bacc("TRN2", target_bir_lowering=False, debug=False)

    xr_d = nc.dram_tensor("xr", [D], F16, kind="ExternalInput")
    xc_d = nc.dram_tensor("xc", [128, KC], F16, kind="ExternalInput")
    wg_d = nc.dram_tensor("wg", [FSH, D], F16, kind="ExternalInput")
    wu_d = nc.dram_tensor("wu", [D, FSH], F16, kind="ExternalInput")
    wd_d = nc.dram_tensor("wd", [FSH, D], F16, kind="ExternalInput")
    thr_d = nc.dram_tensor("thr", [1], F32, kind="ExternalInput")
    out_d = nc.dram_tensor("out", [1, D], F32, kind="ExternalOutput")

    with tile.TileContext(nc) as tc:
        with (
            tc.tile_pool(name="const", bufs=1) as const_pool,
            tc.tile_pool(name="gpool", bufs=8) as gpool,
            tc.tile_pool(name="upool", bufs=6) as upool,
            tc.tile_pool(name="dpool", bufs=4) as dpool,
            tc.tile_pool(name="acts", bufs=1) as acts,
            tc.tile_pool(name="psum", bufs=1, space="PSUM") as psum,
        ):
            # constants on the scalar (qAct) ring so the weight stream on
            # the sync (qSP) ring starts at t=0
            x_rep = const_pool.tile([128, D], F16)
            nc.scalar.dma_start(out=x_rep[:], in_=_bcast(xr_d.ap(), 128))
            xc_sb = const_pool.tile([128, KC], F16)
            nc.scalar.dma_start(out=xc_sb[:], in_=xc_d.ap())
            thr_sb = const_pool.tile([128, 1], F32)
            nc.scalar.dma_start(out=thr_sb[:], in_=_bcast(thr_d.ap(), 128))

            ones16 = const_pool.tile([1, 128], F16)
            nc.vector.memset(ones16[:], 1.0)
            u_sb = const_pool.tile([1, 3 * 512], F16)
            zm16 = const_pool.tile([128, FC], F16)

            x1 = acts.tile([128, FC], F32)   # gate pre-activation accum
            nc.vector.memset(x1[:], 0.0)

            # warm the sigmoid/abs ACT tables while the DMA stream runs
            warm = acts.tile([128, 1], F32)
            nc.scalar.activation(
                warm[:], thr_sb[:], mybir.ActivationFunctionType.Sigmoid
            )
            nc.scalar.activation(
                warm[:], thr_sb[:], mybir.ActivationFunctionType.Abs
            )

            # PSUM tiles.  PE matmul outputs may only start at partition
            # 0/32/64, so the up-GEMV f-tiles live at rows 0/32/64 of one
            # bank and the 8 down-proj d-tiles at rows 0/32/64 of 3 banks.
            u_ps = psum.tile([128, 512], F32)
            nc.vector.memset(u_ps[:], 0.0)
            uT_ps = psum.tile([128, FC], F32)
            nc.vector.memset(uT_ps[:], 0.0)
            o_ps = [
                psum.tile([128, 512], F32, tag=f"o{t}", name=f"o{t}")
                for t in range(3)
            ]

            # gate GEMV on DVE: x1[p, c] = sum_j Wg[c*128+p, j] * x[j]
            for c in range(FC):
                p = 128 if c < FC - 1 else FLAST
                gt = gpool.tile([128, D], F16, tag="g", name="gt")
                nc.sync.dma_start(
                    out=gt[:p, :], in_=wg_d.ap()[c * 128 : c * 128 + p, :]
                )
                nc.vector.affine_mul_reduce(
                    out=gt[:p, :],
                    accum_out=x1[:p, c : c + 1],
                    in0=gt[:p, :],
                    in1=x_rep[:p, :],
                    scale=1.0,
                    bias=0.0,
                )

            # silu + threshold mask, all [128, FC]
            sg = acts.tile([128, FC], F32, tag="sg", name="sg")
            nc.scalar.activation(
                sg[:], x1[:], mybir.ActivationFunctionType.Sigmoid
            )
            x1s = acts.tile([128, FC], F32, tag="x1s", name="x1s")
            nc.vector.tensor_mul(x1s[:], x1[:], sg[:])
            absx = acts.tile([128, FC], F32, tag="absx", name="absx")
            nc.scalar.activation(
                absx[:], x1s[:], mybir.ActivationFunctionType.Abs
            )
            mask = acts.tile([128, FC], F32, tag="mask", name="mask")
            nc.vector.tensor_scalar(
                out=mask[:],
                in0=absx[:],
                scalar1=thr_sb[:],
                scalar2=None,
                op0=mybir.AluOpType.is_gt,
            )
            x1sm = acts.tile([128, FC], F32, tag="x1sm", name="x1sm")
            nc.vector.tensor_mul(x1sm[:], x1s[:], mask[:])

            # up GEMV on PE: u[f] = sum_k x[k] * Wup.T[k, f], f-tiles in PSUM
            for kc in range(KC):
                ut = upool.tile([128, FSH], F16, tag="u", name="ut")
                nc.sync.dma_start(
                    out=ut[:], in_=wu_d.ap()[kc * 128 : (kc + 1) * 128, :]
                )
                for nt, (n0, n1) in enumerate(NTS):
                    nc.tensor.matmul(
                        out=u_ps[32 * nt : 32 * nt + 1, 0 : n1 - n0],
                        lhsT=xc_sb[:, kc : kc + 1],
                        rhs=ut[:, n0:n1],
                        start=(kc == 0),
                        stop=(kc == KC - 1),
                    )

            # u back to [128, FC] chunk-column layout: cast the 3 f-tile rows
            # into one f16 [1, 1376] row, then 11 ones-matmul row->column
            # transposes (f = flat column index)
            for nt, (n0, n1) in enumerate(NTS):
                nc.scalar.copy(
                    u_sb[0:1, n0:n1], u_ps[32 * nt : 32 * nt + 1, 0 : n1 - n0]
                )
            for c in range(FC):
                p = 128 if c < FC - 1 else FLAST
                nc.tensor.matmul(
                    out=uT_ps[0:p, c : c + 1],
                    lhsT=u_sb[0:1, c * 128 : c * 128 + p],
                    rhs=ones16[0:1, 0:1],
                    start=True,
                    stop=True,
                )

            # z = u * silu(x1) * flags, cast to f16 for the PE down-proj
            zm = acts.tile([128, FC], F32, tag="zm", name="zm")
            nc.vector.tensor_mul(zm[:], uT_ps[:], x1sm[:])
            nc.scalar.copy(zm16[:, 0 : FC - 1], zm[:, 0 : FC - 1])
            nc.scalar.copy(zm16[0:FLAST, FC - 1 : FC], zm[0:FLAST, FC - 1 : FC])

            # down projection on PE: o[dt, j] += sum_p zm[c*128+p] * Wd[c*128+p, dt*512+j]
            for c in range(FC):
                p = 128 if c < FC - 1 else FLAST
                dt_ = dpool.tile([128, D], F16, tag="d", name="dt_")
                nc.sync.dma_start(
                    out=dt_[:p, :], in_=wd_d.ap()[c * 128 : c * 128 + p, :]
                )
                for nt in range(NDT):
                    r = 32 * (nt % 3)
                    nc.tensor.matmul(
                        out=o_ps[nt // 3][r : r + 1, :],
                        lhsT=zm16[0:p, c : c + 1],
                        rhs=dt_[0:p, nt * 512 : (nt + 1) * 512],
                        start=(c == 0),
                        stop=(c == FC - 1),
                    )

            # drain PSUM to SBUF split across Act and DVE, then ship the two
            # output halves as soon as their copies land
            osb = const_pool.tile([1, D], F32)
            for nt in range(4):
                r = 32 * (nt % 3)
                if nt % 2 == 0:
                    nc.scalar.copy(
                        osb[0:1, nt * 512 : (nt + 1) * 512],
                        o_ps[nt // 3][r : r + 1, :],
                    )
                else:
                    nc.vector.tensor_copy(
                        out=osb[0:1, nt * 512 : (nt + 1) * 512],
                        in_=o_ps[nt // 3][r : r + 1, :],
                    )
            nc.sync.dma_start(
                out=out_d.ap()[0:1, 0 : 4 * 512], in_=osb[0:1, 0 : 4 * 512]
            )
            for nt in range(4, NDT):
                r = 32 * (nt % 3)
                if nt % 2 == 0:
                    nc.scalar.copy(
                        osb[0:1, nt * 512 : (nt + 1) * 512],
                        o_ps[nt // 3][r : r + 1, :],
                    )
                else:
                    nc.vector.tensor_copy(
                        out=osb[0:1, nt * 512 : (nt + 1) * 512],
                        in_=o_ps[nt // 3][r : r + 1, :],
                    )
            nc.sync.dma_start(
                out=out_d.ap()[0:1, 4 * 512 : D], in_=osb[0:1, 4 * 512 : D]
            )

    nc.compile()
    return nc


def _get_nc():
    if "nc" not in _CACHE:
        _CACHE["nc"] = _build_nc()
    return _CACHE["nc"]


def make_in_maps(x, Wup, Wgatet, Wdownt, threshold):
    """Shard full inputs into the 8 per-core input maps (fp16 weights)."""
    x_flat = np.asarray(x, dtype=np.float32).reshape(D)
    xf = np.ascontiguousarray(x_flat.astype(np.float16))
    xc = np.ascontiguousarray(xf.reshape(KC, 128).T)        # [128, KC]
    thr = np.asarray(threshold, dtype=np.float32).reshape(1)
    Wup = np.asarray(Wup)
    Wgatet = np.asarray(Wgatet)
    Wdownt = np.asarray(Wdownt)
    in_maps = []
    for i in range(NCORES):
        sl = slice(i * FSH, (i + 1) * FSH)
        wg = np.ascontiguousarray(Wgatet[:, sl].T, dtype=np.float16)  # [FSH, D]
        wu = np.ascontiguousarray(Wup[sl, :].T, dtype=np.float16)     # [D, FSH]
        wd = np.ascontiguousarray(Wdownt[sl, :], dtype=np.float16)    # [FSH, D]
        in_maps.append(
            {"xr": xf, "xc": xc, "wg": wg, "wu": wu, "wd": wd, "thr": thr}
        )
    return in_maps


def run_sharded(x, Wup, Wgatet, Wdownt, threshold, trace=False, tmpdir=None):
    """Run on the 8 NeuronCores; returns (full_output, BassKernelResults)."""
    nc = _get_nc()
    in_maps = make_in_maps(x, Wup, Wgatet, Wdownt, threshold)
    res = run_bass_kernel_spmd(
        nc, in_maps, list(range(NCORES)), trace=trace, tmpdir=tmpdir
    )
    # un-shard: out[nt, j] holds partial_out[nt*512 + j]; sum partials
    acc = np.zeros(D, dtype=np.float64)
    for r in res.results:
        acc += r["out"].reshape(D).astype(np.float64)
    out = acc.astype(np.float32).reshape(1, 1, D)
    return out, res


def kernel(x, Wup, Wgatet, Wdownt, threshold):
    out, _ = run_sharded(x, Wup, Wgatet, Wdownt, threshold)
    return out


# revision 21
# speedup vs baseline: 1.9292x; 1.0092x over previous
# CATS-SwiGLU decode kernel for TRN2 (8 NeuronCores, SPMD tensor-parallel).
#
# Reference computation (decode path, B=S=1):
#   x1    = silu(x @ Wgatet)                  [1,1,dff]
#   flags = |x1| > threshold
#   z     = where(flags, (x @ Wup.T) * x1, 0) [1,1,dff]
#   out   = z @ Wdownt                        [1,1,d]
#
# Sharding: d_ff (11008) split across 8 cores (1376 rows each). Each core
# computes its z slice and a full-width partial down-projection; the host
# sums the 8 partials (the all-reduce of the TP hint, done on host).
#
# All three weight matrices are cast to fp16 on the host (rel err ~2.4e-3,
# tolerance 2e-2), halving HBM traffic vs fp32 — the kernel is DMA-bound, so
# this halves runtime.  Engine split so every engine has >=2x slack vs the
# ~98us weight stream:
#   - gate GEMV on DVE: affine_mul_reduce over [128,4096] fp16 chunks
#     (f-rows on partitions) against an on-chip-replicated x; x1 lands
#     directly in [128,11] chunk-column layout.
#   - up GEMV on PE: Wup host-transposed to [4096,1376] (k on partitions),
#     x k-chunks as stationary, fp16 1 cycle/row; accumulates in PSUM.
#   - z-chain ([128,11] elementwise) on DVE/Act; u transposed to the
#     [128,11] layout with 11 tiny ones-matmuls.
#   - down-projection on PE: Wdownt natural [1376,4096] rows-on-partitions
#     chunks as moving tensor, zm fp16 columns as stationary, accumulating
#     [8,512] d-tiles in PSUM.
import sys

for _p in ("/opt/trn_rl_repo",):
    if _p not in sys.path:
        sys.path.insert(0, _p)

import numpy as np

import concourse.bass as bass
import concourse.tile as tile
from concourse import bacc, mybir
from concourse.bass_utils import run_bass_kernel_spmd

D = 4096
FF = 11008
NCORES = 8
FSH = FF // NCORES          # 1376 rows of d_ff per core
FC = (FSH + 127) // 128     # 11 f-chunks of <=128 rows
FLAST = FSH - 128 * (FC - 1)  # 96 rows in the last f-chunk
KC = D // 128               # 32 k-chunks for the up GEMV
NTS = ((0, 512), (512, 1024), (1024, FSH))  # f n-tiles for up-GEMV PSUM rows
NDT = 8                     # down-proj d-tiles of 512
F32 = mybir.dt.float32
F16 = mybir.dt.float16

_CACHE = {}


def _bcast(ap, parts):
    """Replicate a 1-D AP across `parts` partitions (0-stride partition dim)."""
    return bass.AP(tensor=ap.tensor, offset=ap.offset, ap=[[0, parts]] + list(ap.ap))


def _build_nc():
    nc = bacc.Bacc("TRN2", target_bir_lowering=False, debug=False)

    xr_d = nc.dram_tensor("xr", [D], F16, kind="ExternalInput")
    xc_d = nc.dram_tensor("xc", [128, KC], F16, kind="ExternalInput")
    wg_d = nc.dram_tensor("wg", [FSH, D], F16, kind="ExternalInput")
    wu_d = nc.dram_tensor("wu", [D, FSH], F16, kind="ExternalInput")
    wd_d = nc.dram_tensor("wd", [FSH, D], F16, kind="ExternalInput")
    thr_d = nc.dram_tensor("thr", [1], F32, kind="ExternalInput")
    out_d = nc.dram_tensor("out", [1, D], F32, kind="ExternalOutput")

    with tile.TileContext(nc) as tc:
        with (
            tc.tile_pool(name="const", bufs=1) as const_pool,
            tc.tile_pool(name="gpool", bufs=8) as gpool,
            tc.tile_pool(name="upool", bufs=3) as upool,
            tc.tile_pool(name="dpool", bufs=3) as dpool,
            tc.tile_pool(name="acts", bufs=1) as acts,
            tc.tile_pool(name="psum", bufs=1, space="PSUM") as psum,
        ):
            # constants on the scalar (qAct) ring so the weight stream on
            # the sync (qSP) ring starts at t=0
            x_rep = const_pool.tile([128, D], F16)
            nc.scalar.dma_start(out=x_rep[:], in_=_bcast(xr_d.ap(), 128))
            xc_sb = const_pool.tile([128, KC], F16)
            nc.scalar.dma_start(out=xc_sb[:], in_=xc_d.ap())
            thr_sb = const_pool.tile([128, 1], F32)
            nc.scalar.dma_start(out=thr_sb[:], in_=_bcast(thr_d.ap(), 128))

            ones16 = const_pool.tile([1, 128], F16)
            nc.vector.memset(ones16[:], 1.0)
            u_sb = const_pool.tile([1, 3 * 512], F16)
            zm16 = const_pool.tile([128, FC], F16)

            x1 = acts.tile([128, FC], F32)   # gate pre-activation accum
            nc.vector.memset(x1[:], 0.0)

            # warm the sigmoid/abs ACT tables while the DMA stream runs
            warm = acts.tile([128, 1], F32)
            nc.scalar.activation(
                warm[:], thr_sb[:], mybir.ActivationFunctionType.Sigmoid
            )
            nc.scalar.activation(
                warm[:], thr_sb[:], mybir.ActivationFunctionType.Abs
            )

            # PSUM tiles.  PE matmul outputs may only start at partition
            # 0/32/64, so the up-GEMV f-tiles live at rows 0/32/64 of one
            # bank and the 8 down-proj d-tiles at rows 0/32/64 of 3 banks.
            u_ps = psum.tile([128, 512], F32)
            nc.vector.memset(u_ps[:], 0.0)
            uT_ps = psum.tile([128, FC], F32)
            nc.vector.memset(uT_ps[:], 0.0)
            o_ps = [
                psum.tile([128, 512], F32, tag=f"o{t}", name=f"o{t}")
                for t in range(3)
            ]

            # gate GEMV on DVE: x1[p, c] = sum_j Wg[c*128+p, j] * x[j]
            for c in range(FC):
                p = 128 if c < FC - 1 else FLAST
                gt = gpool.tile([128, D], F16, tag="g", name="gt")
                nc.sync.dma_start(
                    out=gt[:p, :], in_=wg_d.ap()[c * 128 : c * 128 + p, :]
                )
                nc.vector.affine_mul_reduce(
                    out=gt[:p, :],
                    accum_out=x1[:p, c : c + 1],
                    in0=gt[:p, :],
                    in1=x_rep[:p, :],
                    scale=1.0,
                    bias=0.0,
                )

            # silu + threshold mask, all [128, FC]
            sg = acts.tile([128, FC], F32, tag="sg", name="sg")
            nc.scalar.activation(
                sg[:], x1[:], mybir.ActivationFunctionType.Sigmoid
            )
            x1s = acts.tile([128, FC], F32, tag="x1s", name="x1s")
            nc.vector.tensor_mul(x1s[:], x1[:], sg[:])
            absx = acts.tile([128, FC], F32, tag="absx", name="absx")
            nc.scalar.activation(
                absx[:], x1s[:], mybir.ActivationFunctionType.Abs
            )
            mask = acts.tile([128, FC], F32, tag="mask", name="mask")
            nc.vector.tensor_scalar(
                out=mask[:],
                in0=absx[:],
                scalar1=thr_sb[:],
                scalar2=None,
                op0=mybir.AluOpType.is_gt,
            )
            x1sm = acts.tile([128, FC], F32, tag="x1sm", name="x1sm")
            nc.vector.tensor_mul(x1sm[:], x1s[:], mask[:])

            # up GEMV on PE: u[f] = sum_k x[k] * Wup.T[k, f], f-tiles in PSUM.
            # 4 k-chunks ride in one DMA ([128, 4, FSH] 3-D access pattern).
            KM = 4
            for kb in range(KC // KM):
                ut = upool.tile([128, KM * FSH], F16, tag="u", name="ut")
                src = wu_d.ap()[kb * KM * 128 : (kb + 1) * KM * 128, :]
                nc.sync.dma_start(
                    out=ut[:],
                    in_=bass.AP(
                        tensor=src.tensor,
                        offset=src.offset,
                        ap=[[FSH, 128], [128 * FSH, KM], [1, FSH]],
                    ),
                )
                for km in range(KM):
                    kc = kb * KM + km
                    for nt, (n0, n1) in enumerate(NTS):
                        nc.tensor.matmul(
                            out=u_ps[32 * nt : 32 * nt + 1, 0 : n1 - n0],
                            lhsT=xc_sb[:, kc : kc + 1],
                            rhs=ut[:, km * FSH + n0 : km * FSH + n1],
                            start=(kc == 0),
                            stop=(kc == KC - 1),
                        )

            # u back to [128, FC] chunk-column layout: cast the 3 f-tile rows
            # into one f16 [1, 1376] row (Act + DVE in parallel), then 11
            # ones-matmul row->column transposes (f = flat column index)
            nc.vector.tensor_copy(
                out=u_sb[0:1, NTS[2][0] : NTS[2][1]],
                in_=u_ps[64:65, 0 : NTS[2][1] - NTS[2][0]],
            )
            for nt, (n0, n1) in enumerate(NTS[:2]):
                nc.scalar.copy(
                    u_sb[0:1, n0:n1], u_ps[32 * nt : 32 * nt + 1, 0 : n1 - n0]
                )
            for c in range(FC):
                p = 128 if c < FC - 1 else FLAST
                nc.tensor.matmul(
                    out=uT_ps[0:p, c : c + 1],
                    lhsT=u_sb[0:1, c * 128 : c * 128 + p],
                    rhs=ones16[0:1, 0:1],
                    start=True,
                    stop=True,
                )

            # z = u * silu(x1) * flags, cast to f16 for the PE down-proj
            zm = acts.tile([128, FC], F32, tag="zm", name="zm")
            nc.vector.tensor_mul(zm[:], uT_ps[:], x1sm[:])
            nc.scalar.copy(zm16[:, 0 : FC - 1], zm[:, 0 : FC - 1])
            nc.vector.tensor_copy(
                out=zm16[0:FLAST, FC - 1 : FC], in_=zm[0:FLAST, FC - 1 : FC]
            )

            # down projection on PE: o[dt, j] += sum_p zm[c*128+p] * Wd[c*128+p, dt*512+j]
            # 2 f-chunks ride in one DMA ([128, 2, D] 3-D access pattern).
            for cb in range((FC + 1) // 2):
                c0, c1 = 2 * cb, min(2 * cb + 2, FC)
                dt_ = dpool.tile([128, 2 * D], F16, tag="d", name="dt_")
                src = wd_d.ap()[c0 * 128 :, :]
                if c1 - c0 == 2:
                    nc.sync.dma_start(
                        out=dt_[:],
                        in_=bass.AP(
                            tensor=src.tensor,
                            offset=src.offset,
                            ap=[[D, 128], [128 * D, 2], [1, D]],
                        ),
                    )
                else:
                    nc.sync.dma_start(
                        out=dt_[:FLAST, 0:D],
                        in_=wd_d.ap()[c0 * 128 : c0 * 128 + FLAST, :],
                    )
                for c in range(c0, c1):
                    p = 128 if c < FC - 1 else FLAST
                    off = (c - c0) * D
                    for nt in range(NDT):
                        r = 32 * (nt % 3)
                        nc.tensor.matmul(
                            out=o_ps[nt // 3][r : r + 1, :],
                            lhsT=zm16[0:p, c : c + 1],
                            rhs=dt_[0:p, off + nt * 512 : off + (nt + 1) * 512],
                            start=(c == 0),
                            stop=(c == FC - 1),
                        )

            # drain PSUM to SBUF split across Act and DVE, then ship the two
            # output halves as soon as their copies land
            osb = const_pool.tile([1, D], F32)
            for nt in range(4):
                r = 32 * (nt % 3)
                if nt % 2 == 0:
                    nc.scalar.copy(
                        osb[0:1, nt * 512 : (nt + 1) * 512],
                        o_ps[nt // 3][r : r + 1, :],
                    )
                else:
                    nc.vector.tensor_copy(
                        out=osb[0:1, nt * 512 : (nt + 1) * 512],
                        in_=o_ps[nt // 3][r : r + 1, :],
                    )
            nc.sync.dma_start(
                out=out_d.ap()[0:1, 0 : 4 * 512], in_=osb[0:1, 0 : 4 * 512]
            )
            for nt in range(4, NDT):
                r = 32 * (nt % 3)
                if nt % 2 == 0:
                    nc.scalar.copy(
                        osb[0:1, nt * 512 : (nt + 1) * 512],
                        o_ps[nt // 3][r : r + 1, :],
                    )
                else:
                    nc.vector.tensor_copy(
                        out=osb[0:1, nt * 512 : (nt + 1) * 512],
                        in_=o_ps[nt // 3][r : r + 1, :],
                    )
            nc.sync.dma_start(
                out=out_d.ap()[0:1, 4 * 512 : D], in_=osb[0:1, 4 * 512 : D]
            )

    nc.compile()
    return nc


def _get_nc():
    if "nc" not in _CACHE:
        _CACHE["nc"] = _build_nc()
    return _CACHE["nc"]


def make_in_maps(x, Wup, Wgatet, Wdownt, threshold):
    """Shard full inputs into the 8 per-core input maps (fp16 weights)."""
    x_flat = np.asarray(x, dtype=np.float32).reshape(D)
    xf = np.ascontiguousarray(x_flat.astype(np.float16))
    xc = np.ascontiguousarray(xf.reshape(KC, 128).T)        # [128, KC]
    thr = np.asarray(threshold, dtype=np.float32).reshape(1)
    Wup = np.asarray(Wup)
    Wgatet = np.asarray(Wgatet)
    Wdownt = np.asarray(Wdownt)
    in_maps = []
    for i in range(NCORES):
        sl = slice(i * FSH, (i + 1) * FSH)
        wg = np.ascontiguousarray(Wgatet[:, sl].T, dtype=np.float16)  # [FSH, D]
        wu = np.ascontiguousarray(Wup[sl, :].T, dtype=np.float16)     # [D, FSH]
        wd = np.ascontiguousarray(Wdownt[sl, :], dtype=np.float16)    # [FSH, D]
        in_maps.append(
            {"xr": xf, "xc": xc, "wg": wg, "wu": wu, "wd": wd, "thr": thr}
        )
    return in_maps


def run_sharded(x, Wup, Wgatet, Wdownt, threshold, trace=False, tmpdir=None):
    """Run on the 8 NeuronCores; returns (full_output, BassKernelResults)."""
    nc = _get_nc()
    in_maps = make_in_maps(x, Wup, Wgatet, Wdownt, threshold)
    res = run_bass_kernel_spmd(
        nc, in_maps, list(range(NCORES)), trace=trace, tmpdir=tmpdir
    )
    # un-shard: out[nt, j] holds partial_out[nt*512 + j]; sum partials
    acc = np.zeros(D, dtype=np.float64)
    for r in res.results:
        acc += r["out"].reshape(D).astype(np.float64)
    out = acc.astype(np.float32).reshape(1, 1, D)
    return out, res


def kernel(x, Wup, Wgatet, Wdownt, threshold):
    out, _ = run_sharded(x, Wup, Wgatet, Wdownt, threshold)
    return out
